# revision 31
# baseline (speedup 1.0000x reference)
"""LocalitySelfAttention TRN2 kernel.

B=4, N=2048, C=768, H=12, D=64.  8 cores: core c -> batch c//2, heads
6*(c%2) .. 6*(c%2)+6 (6 contiguous heads).  Each core computes its heads'
qkv projection, attention (scores kept transposed: [keys, qrows] so softmax
sums come from a fused ones-column in the AV matmul), and a partial output
projection restricted to its heads' 384 rows of w_proj.  Host sums the two
partials per batch and adds b_proj.

All-transposed dataflow: host passes x[b].T in bf16; q/k are produced
transposed ([64, 2048] per head, stationary = w_qkv columns), v natural
([2048, 64], stationary = xT blocks).  ST block = kT_blk.T @ qT ->
[128 keys, qrows]; exp on ACT with scale=0.125; AV: lhsT = v_aug
[keys, 64+1(ones)], rhs = PT -> outT_aug [65, qrows] accumulated over key
blocks; row 64 = softmax sums.  Diagonal temperature factor: one [128,128]
mask multiply per (head, kblock) on the diagonal sub-block before exp.

Single shared PSUM work pool (3 x 4KB slots) + one AV accumulator slot so
the Tile scheduler can overlap the qkv matmuls of head-pair p+1 and the
output projection with the ACT(exp)-bound attention inner loop.  The exp
on the scalar engine (192 x [128,1024] tiles) is the hard floor; every
other engine's work is arranged to hide under it.
"""

import sys
import numpy as np

if "/opt/trn_rl_repo" not in sys.path:
    sys.path.insert(0, "/opt/trn_rl_repo")

B, N, C, H = 4, 2048, 768, 12
D = C // H          # 64
NH = 6              # heads per core
P = 128
CT = C // P         # 6 contraction tiles
KB = N // P         # 16 key blocks
QC = N // 512       # 4 free-dim chunks of 512
SCALE = float(D) ** -0.5  # 0.125

_CACHE = {}


def _build_program():
    import concourse.bass as bass
    import concourse.mybir as mybir
    import concourse.tile as tile
    from concourse import bacc
    from concourse.masks import make_identity

    f32 = mybir.dt.float32
    bf16 = mybir.dt.bfloat16
    Exp = mybir.ActivationFunctionType.Exp
    mult = mybir.AluOpType.mult
    add = mybir.AluOpType.add

    nc = bacc.Bacc()
    xT = nc.dram_tensor("xT", [C, N], bf16, kind="ExternalInput")
    wqkv = nc.dram_tensor("wqkv", [C, 3 * NH * D], bf16, kind="ExternalInput")
    wproj = nc.dram_tensor("wproj", [NH * D, C], bf16, kind="ExternalInput")
    temp = nc.dram_tensor("temp", [P, NH], f32, kind="ExternalInput")
    outT = nc.dram_tensor("outT", [C, N], bf16, kind="ExternalOutput")
    rdram = nc.dram_tensor("rscratch", [NH, N], f32)  # recip-row bounce

    HF = N // 2  # 1024-column halves
    G3 = NH * D // P  # 3 row-groups of w_proj

    def mm(out, lhsT, rhs, **kw):
        nc.tensor.matmul(out, lhsT, rhs, **kw)

    with tile.TileContext(nc) as tc:
        with (
            tc.tile_pool(name="const", bufs=1) as constp,
            tc.tile_pool(name="persist", bufs=1) as persist,
            tc.tile_pool(name="pwork", bufs=3, space=bass.MemorySpace.PSUM) as pwork,
            tc.tile_pool(name="pav", bufs=1, space=bass.MemorySpace.PSUM) as pav,
            tc.tile_pool(name="pt", bufs=6) as ptp,
            tc.tile_pool(name="rb", bufs=2) as rbp,
            tc.tile_pool(name="un", bufs=2) as unp,
            tc.tile_pool(name="ot", bufs=2) as otp,
        ):
            # ---- inputs ------------------------------------------------
            # inputs: xts on the SP DMA queue, weights on the ACT DMA queue
            # so the first v matmul's operands land in parallel.
            xts, wqs = [], []
            for t in range(CT):
                xti = persist.tile([P, N], bf16, tag=f"xt{t}")
                xts.append(xti)
                wqi = persist.tile([P, 3 * NH * D], bf16, tag=f"wq{t}")
                nc.scalar.dma_start(wqi[:], wqkv[t * P : (t + 1) * P, :])
                wqs.append(wqi)
            # x loads split in column halves, low half first: the first half
            # of the v blocks (and qk half-tiles) can start before the high
            # halves land.
            for half in range(2):
                for t in range(CT):
                    nc.sync.dma_start(
                        xts[t][:, half * HF : (half + 1) * HF],
                        xT[t * P : (t + 1) * P, half * HF : (half + 1) * HF],
                    )
            wp = persist.tile([P, G3, C], bf16, tag="wp")
            for g3 in range(G3):
                nc.scalar.dma_start(wp[:, g3, :], wproj[g3 * P : (g3 + 1) * P, :])

            # ---- setup: temperature diag masks (1 - t_h * I) -----------
            ident = constp.tile([P, P], f32, tag="ident")
            make_identity(nc, ident[:])
            tbc = constp.tile([P, NH], f32, tag="tbc")
            nc.sync.dma_start(tbc[:, :], temp[:, :])
            ntb = constp.tile([P, NH], f32, tag="ntb")
            nc.vector.tensor_scalar_mul(ntb[:, :], tbc[:, :], -1.0)
            masks = constp.tile([P, NH, P], f32, tag="masks")
            for h in range(NH):
                nc.vector.tensor_scalar(
                    masks[:, h, :], ident[:], ntb[:, h : h + 1], 1.0, mult, add
                )

            # persistent: qT/kT (bf16, transposed) and v_aug with ones col
            qkT = persist.tile([P, 2 * NH, N], bf16, tag="qkT")  # 0-2 q, 3-5 k
            vaug = persist.tile([P, KB, NH, D + 1], bf16, tag="vaug")
            onesrc = constp.tile([P, KB * NH], f32, tag="onesrc")
            nc.vector.memset(onesrc[:], 1.0)
            # zero stationary for PE keep-warm dummies (see attention loop)
            zt = constp.tile([P, D + 1], bf16, tag="zt")
            nc.vector.memset(zt[:], 0.0)
            nc.vector.tensor_copy(
                vaug[:, :, :, D : D + 1],
                onesrc[:].rearrange("p (a b c) -> p a b c", a=KB, b=NH),
            )
            attnT = persist.tile([P, G3, N], bf16, tag="attnT")

            # ---- v projection (all heads), then q/k transposed ---------
            # All in the shared ps ring (bufs=3) so MMs pipeline against the
            # PSUM->SBUF copies, which alternate DVE/ACT (ACT is idle until
            # the first exp).
            for rb_i in range(KB):
                psv = pwork.tile([P, NH * D], f32, tag="ps", name=f"psv{rb_i}")
                for t in range(CT):
                    mm(
                        psv[:],
                        xts[t][:, rb_i * P : (rb_i + 1) * P],
                        wqs[t][:, 2 * NH * D : 3 * NH * D],
                        start=(t == 0),
                        stop=(t == CT - 1),
                    )
                (nc.vector.tensor_copy if rb_i % 2 else nc.scalar.copy)(
                    vaug[:, rb_i, :, 0:D],
                    psv[:].rearrange("p (h d) -> p h d", h=NH),
                )
            # head-pair 0's q/k groups LAST-but-first: emit q0/k0 first so
            # the first attention scores' inputs are ready the moment the
            # PE reaches them, keeping the qkv->attention transition
            # seamless in the in-order queue.
            for grp in (0, 3, 1, 4, 2, 5):
                for half in range(2):
                    ps = pwork.tile([P, HF], f32, tag="ps",
                                    name=f"qk{grp}_{half}")
                    for t in range(CT):
                        for qc in range(2):
                            mm(
                                ps[:, qc * 512 : (qc + 1) * 512],
                                wqs[t][:, grp * P : (grp + 1) * P],
                                xts[t][:, half * HF + qc * 512 : half * HF + (qc + 1) * 512],
                                start=(t == 0),
                                stop=(t == CT - 1),
                            )
                    dst = qkT[:, grp, half * HF : (half + 1) * HF]
                    ((nc.scalar.copy if (grp + half) % 2 else
                      nc.vector.tensor_copy))(dst, ps[:])

            # ---- attention, head pair by head pair ---------------------
            for p in range(3):
                for hi in range(2):
                    h = 2 * p + hi
                    g = p
                    off = hi * D
                    for hf in range(2):
                        av = pav.tile([D + 1, HF], f32, tag="av",
                                      name=f"av{h}_{hf}")
                        for kb in range(KB):
                            st = pwork.tile([P, HF], f32, tag="ps", name="st")
                            for qc in range(2):
                                mm(
                                    st[:, qc * 512 : (qc + 1) * 512],
                                    qkT[off : off + D, 3 + g, kb * P : (kb + 1) * P],
                                    qkT[off : off + D, g, hf * HF + qc * 512 : hf * HF + (qc + 1) * 512],
                                    start=True,
                                    stop=True,
                                )
                            if kb * P // HF == hf:
                                dcol = kb * P - hf * HF
                                nc.vector.tensor_mul(
                                    st[:, dcol : dcol + P],
                                    st[:, dcol : dcol + P],
                                    masks[:, h, :],
                                )
                            pt = ptp.tile([P, HF], bf16, tag="pt")
                            nc.scalar.activation(pt[:], st[:], Exp, scale=SCALE)
                            for qc in range(2):
                                mm(
                                    av[:, qc * 512 : (qc + 1) * 512],
                                    vaug[:, kb, h, :],
                                    pt[:, qc * 512 : (qc + 1) * 512],
                                    start=(kb == 0),
                                    stop=(kb == KB - 1),
                                )
                            # PE keep-warm: the attention loop alone is
                            # ACT(exp)-bound at ~80% PE utilization, which
                            # lets the tensor engine drop out of its high
                            # p-state (observed 1.6x slower MMs).  Once the
                            # scheduler's hoistable qkv work is exhausted
                            # (from head 2 on), burn the slack with an
                            # exact no-op: av += 0^T @ pt.
                            if p >= 1 and 1 <= kb <= 14:
                                mm(
                                    av[:, 0:192],
                                    zt[:],
                                    pt[:, 0:192],
                                    start=False,
                                    stop=False,
                                )
                        # normalize: rows 0..63 * recip(row 64).  In-place
                        # single-lane reciprocal + SBUF-source broadcast DMA
                        # keeps the chain short (it gates the proj start via
                        # the last head).
                        un = unp.tile([P, HF], f32, tag="un")
                        nc.vector.tensor_copy(un[0 : D + 1, :], av[:])
                        nc.vector.reciprocal(un[D : D + 1, :], un[D : D + 1, :])
                        nc.sync.dma_start(
                            rdram[h, hf * HF : (hf + 1) * HF], un[D : D + 1, :]
                        )
                        rb = rbp.tile([P, HF], f32, tag="rb")
                        nc.sync.dma_start(
                            rb[0:D, :],
                            rdram[h : h + 1, hf * HF : (hf + 1) * HF]
                            .broadcast_to([D, HF]),
                        )
                        nc.vector.tensor_mul(
                            attnT[off : off + D, g, hf * HF : (hf + 1) * HF],
                            un[0:D, :],
                            rb[0:D, :],
                        )

            # ---- output projection (transposed) ------------------------
            for m in range(CT):
                ot = otp.tile([P, N], bf16, tag="ot")
                for half in range(2):
                    po = pwork.tile([P, HF], f32, tag="ps", name=f"po{m}_{half}")
                    for g3 in range(G3):
                        for qc in range(2):
                            sl = slice(half * HF + qc * 512,
                                       half * HF + (qc + 1) * 512)
                            psl = slice(qc * 512, (qc + 1) * 512)
                            mm(
                                po[:, psl],
                                wp[:, g3, m * P : (m + 1) * P],
                                attnT[:, g3, sl],
                                start=(g3 == 0),
                                stop=(g3 == G3 - 1),
                            )
                    ((nc.scalar.copy if (m + half) % 2 else
                      nc.vector.tensor_copy))(
                        ot[:, half * HF : (half + 1) * HF], po[:]
                    )
                    # stream each output half out as soon as it's copied so
                    # the final DMA tail is ~0.5 MB, not 6 MB.  On the ACT
                    # hwdge queue: the sync queue carries the normalize
                    # broadcasts that gate the proj start.
                    nc.scalar.dma_start(
                        outT[m * P : (m + 1) * P, half * HF : (half + 1) * HF],
                        ot[:, half * HF : (half + 1) * HF],
                    )

    if not nc.is_finalized():
        nc.finalize()
    return nc


def _get_program():
    if "nc" not in _CACHE:
        _CACHE["nc"] = _build_program()
    return _CACHE["nc"]


def _in_maps(x, w_qkv, w_proj, temperature):
    import ml_dtypes

    bf = ml_dtypes.bfloat16
    t = np.asarray(temperature, dtype=np.float32).reshape(H)
    maps = []
    xTs = {}
    for c in range(8):
        b, h0 = c // 2, NH * (c % 2)
        if b not in xTs:
            xTs[b] = np.ascontiguousarray(
                np.asarray(x[b], dtype=np.float32).T
            ).astype(bf)
        cols = slice(D * h0, D * h0 + NH * D)
        wq = np.concatenate(
            [w_qkv[:, cols], w_qkv[:, C:][:, cols], w_qkv[:, 2 * C :][:, cols]],
            axis=1,
        )
        maps.append(
            {
                "xT": xTs[b],
                "wqkv": np.ascontiguousarray(wq, dtype=np.float32).astype(bf),
                "wproj": np.ascontiguousarray(
                    w_proj[D * h0 : D * h0 + NH * D, :], dtype=np.float32
                ).astype(bf),
                "temp": np.ascontiguousarray(
                    np.broadcast_to(t[h0 : h0 + NH].reshape(1, NH), (P, NH))
                ),
            }
        )
    return maps


def _install_profile_hook():
    """The agent image's antenv lacks axon_hooks; synthesize it and register
    the ctypes NTFF hook so run_bass_kernel_spmd(trace=True) can profile."""
    import types, importlib

    if "antenv.axon_hooks" not in sys.modules:
        import antenv

        mod = types.ModuleType("antenv.axon_hooks")
        _state = {"hook": None}
        mod.set_axon_ntff_profile_hook = lambda h: _state.__setitem__("hook", h)
        mod.get_axon_ntff_profile_hook = lambda: _state["hook"]
        sys.modules["antenv.axon_hooks"] = mod
        antenv.axon_hooks = mod
    from antenv.axon_hooks import (
        get_axon_ntff_profile_hook,
        set_axon_ntff_profile_hook,
    )

    if get_axon_ntff_profile_hook() is None:
        tb = importlib.import_module("trn_agent_boot.trn_boot")
        hook = tb._ntff_profile_via_ctypes("/opt/axon/libaxon_pjrt.so")
        set_axon_ntff_profile_hook(hook)


def kernel(x, w_qkv, w_proj, b_proj, temperature, _trace=False):
    from concourse.bass_utils import run_bass_kernel_spmd

    if _trace:
        try:
            _install_profile_hook()
        except Exception as e:  # profiling is best-effort
            print(f"profile hook install failed: {e}")

    nc = _get_program()
    maps = _in_maps(
        np.asarray(x, np.float32),
        np.asarray(w_qkv, np.float32),
        np.asarray(w_proj, np.float32),
        np.asarray(temperature, np.float32),
    )
    res = run_bass_kernel_spmd(nc, maps, list(range(8)), trace=_trace)
    parts = [np.asarray(r["outT"]).astype(np.float32) for r in res.results]
    bp = np.asarray(b_proj, np.float32)
    out = np.stack(
        [(parts[2 * b] + parts[2 * b + 1]).T + bp for b in range(B)]
    ).astype(np.float32)
    if _trace:
        _CACHE["last_result"] = res
    return out


# revision 33
# speedup vs baseline: 1.2409x; 1.2409x over previous
"""LocalitySelfAttention TRN2 kernel.

B=4, N=2048, C=768, H=12, D=64.  8 cores: core c -> batch c//2, heads
6*(c%2) .. 6*(c%2)+6 (6 contiguous heads).  Each core computes its heads'
qkv projection, attention (scores kept transposed: [keys, qrows] so softmax
sums come from a fused ones-column in the AV matmul), and a partial output
projection restricted to its heads' 384 rows of w_proj.  Host sums the two
partials per batch and adds b_proj.

All-transposed dataflow: host passes x[b].T in bf16; q/k are produced
transposed ([64, 2048] per head, stationary = w_qkv columns), v natural
([2048, 64], stationary = xT blocks).  ST block = kT_blk.T @ qT ->
[128 keys, qrows]; exp on ACT with scale=0.125; AV: lhsT = v_aug
[keys, 64+1(ones)], rhs = PT -> outT_aug [65, qrows] accumulated over key
blocks; row 64 = softmax sums.  Diagonal temperature factor: one [128,128]
mask multiply per (head, kblock) on the diagonal sub-block before exp.

Single shared PSUM work pool (3 x 4KB slots) + one AV accumulator slot so
the Tile scheduler can overlap the qkv matmuls of head-pair p+1 and the
output projection with the ACT(exp)-bound attention inner loop.  The exp
on the scalar engine (192 x [128,1024] tiles) is the hard floor; every
other engine's work is arranged to hide under it.
"""

import sys
import numpy as np

if "/opt/trn_rl_repo" not in sys.path:
    sys.path.insert(0, "/opt/trn_rl_repo")

B, N, C, H = 4, 2048, 768, 12
D = C // H          # 64
NH = 6              # heads per core
P = 128
CT = C // P         # 6 contraction tiles
KB = N // P         # 16 key blocks
QC = N // 512       # 4 free-dim chunks of 512
SCALE = float(D) ** -0.5  # 0.125

_CACHE = {}


def _build_program():
    import concourse.bass as bass
    import concourse.mybir as mybir
    import concourse.tile as tile
    from concourse import bacc
    from concourse.masks import make_identity

    f32 = mybir.dt.float32
    bf16 = mybir.dt.bfloat16
    Exp = mybir.ActivationFunctionType.Exp
    mult = mybir.AluOpType.mult
    add = mybir.AluOpType.add

    nc = bacc.Bacc()
    xT = nc.dram_tensor("xT", [C, N], bf16, kind="ExternalInput")
    wqkv = nc.dram_tensor("wqkv", [C, 3 * NH * D], bf16, kind="ExternalInput")
    wproj = nc.dram_tensor("wproj", [NH * D, C], bf16, kind="ExternalInput")
    temp = nc.dram_tensor("temp", [P, NH], f32, kind="ExternalInput")
    outT = nc.dram_tensor("outT", [C, N], bf16, kind="ExternalOutput")
    rdram = nc.dram_tensor("rscratch", [NH, N], f32)  # sum rows bounce
    rdram2 = nc.dram_tensor("rscratch2", [NH, N], f32)  # recip rows bounce

    HF = N // 2  # 1024-column halves
    G3 = NH * D // P  # 3 row-groups of w_proj

    def mm(out, lhsT, rhs, **kw):
        nc.tensor.matmul(out, lhsT, rhs, **kw)

    with tile.TileContext(nc) as tc:
        with (
            tc.tile_pool(name="const", bufs=1) as constp,
            tc.tile_pool(name="persist", bufs=1) as persist,
            tc.tile_pool(name="pwork", bufs=3, space=bass.MemorySpace.PSUM) as pwork,
            tc.tile_pool(name="pav", bufs=1, space=bass.MemorySpace.PSUM) as pav,
            tc.tile_pool(name="pt", bufs=6) as ptp,
            tc.tile_pool(name="rb", bufs=2) as rbp,
            tc.tile_pool(name="un", bufs=2) as unp,
            tc.tile_pool(name="ot", bufs=2) as otp,
        ):
            # ---- inputs ------------------------------------------------
            # inputs: xts on the SP DMA queue, weights on the ACT DMA queue
            # so the first v matmul's operands land in parallel.
            xts, wqs = [], []
            for t in range(CT):
                xti = persist.tile([P, N], bf16, tag=f"xt{t}")
                xts.append(xti)
                wqi = persist.tile([P, 3 * NH * D], bf16, tag=f"wq{t}")
                nc.scalar.dma_start(wqi[:], wqkv[t * P : (t + 1) * P, :])
                wqs.append(wqi)
            # x loads split in column halves, low half first: the first half
            # of the v blocks (and qk half-tiles) can start before the high
            # halves land.
            for half in range(2):
                for t in range(CT):
                    nc.sync.dma_start(
                        xts[t][:, half * HF : (half + 1) * HF],
                        xT[t * P : (t + 1) * P, half * HF : (half + 1) * HF],
                    )
            wp = persist.tile([P, G3, C], bf16, tag="wp")
            for g3 in range(G3):
                nc.scalar.dma_start(wp[:, g3, :], wproj[g3 * P : (g3 + 1) * P, :])

            # ---- setup: temperature diag masks (1 - t_h * I) -----------
            ident = constp.tile([P, P], f32, tag="ident")
            make_identity(nc, ident[:])
            tbc = constp.tile([P, NH], f32, tag="tbc")
            nc.sync.dma_start(tbc[:, :], temp[:, :])
            ntb = constp.tile([P, NH], f32, tag="ntb")
            nc.vector.tensor_scalar_mul(ntb[:, :], tbc[:, :], -1.0)
            masks = constp.tile([P, NH, P], f32, tag="masks")
            for h in range(NH):
                nc.vector.tensor_scalar(
                    masks[:, h, :], ident[:], ntb[:, h : h + 1], 1.0, mult, add
                )

            # persistent: qT/kT (bf16, transposed) and v_aug with ones col
            qkT = persist.tile([P, 2 * NH, N], bf16, tag="qkT")  # 0-2 q, 3-5 k
            vaug = persist.tile([P, KB, NH, D + 1], bf16, tag="vaug")
            onesrc = constp.tile([P, KB * NH], f32, tag="onesrc")
            nc.vector.memset(onesrc[:], 1.0)
            # zero stationary for PE keep-warm dummies (see attention loop)
            zt = constp.tile([P, D + 1], bf16, tag="zt")
            nc.vector.memset(zt[:], 0.0)
            nc.vector.tensor_copy(
                vaug[:, :, :, D : D + 1],
                onesrc[:].rearrange("p (a b c) -> p a b c", a=KB, b=NH),
            )
            attnT = persist.tile([P, G3, N], bf16, tag="attnT")

            # ---- v projection (all heads), then q/k transposed ---------
            # All in the shared ps ring (bufs=3) so MMs pipeline against the
            # PSUM->SBUF copies, which alternate DVE/ACT (ACT is idle until
            # the first exp).
            for rb_i in range(KB):
                psv = pwork.tile([P, NH * D], f32, tag="ps", name=f"psv{rb_i}")
                for t in range(CT):
                    mm(
                        psv[:],
                        xts[t][:, rb_i * P : (rb_i + 1) * P],
                        wqs[t][:, 2 * NH * D : 3 * NH * D],
                        start=(t == 0),
                        stop=(t == CT - 1),
                    )
                (nc.vector.tensor_copy if rb_i % 2 else nc.scalar.copy)(
                    vaug[:, rb_i, :, 0:D],
                    psv[:].rearrange("p (h d) -> p h d", h=NH),
                )
            # head-pair 0's q/k groups LAST-but-first: emit q0/k0 first so
            # the first attention scores' inputs are ready the moment the
            # PE reaches them, keeping the qkv->attention transition
            # seamless in the in-order queue.
            for grp in (0, 3, 1, 4, 2, 5):
                for half in range(2):
                    ps = pwork.tile([P, HF], f32, tag="ps",
                                    name=f"qk{grp}_{half}")
                    for t in range(CT):
                        for qc in range(2):
                            mm(
                                ps[:, qc * 512 : (qc + 1) * 512],
                                wqs[t][:, grp * P : (grp + 1) * P],
                                xts[t][:, half * HF + qc * 512 : half * HF + (qc + 1) * 512],
                                start=(t == 0),
                                stop=(t == CT - 1),
                            )
                    dst = qkT[:, grp, half * HF : (half + 1) * HF]
                    ((nc.scalar.copy if (grp + half) % 2 else
                      nc.vector.tensor_copy))(dst, ps[:])

            # ---- attention, head pair by head pair ---------------------
            for p in range(3):
                for hi in range(2):
                    h = 2 * p + hi
                    g = p
                    off = hi * D
                    for hf in range(2):
                        av = pav.tile([D + 1, HF], f32, tag="av",
                                      name=f"av{h}_{hf}")
                        for kb in range(KB):
                            st = pwork.tile([P, HF], f32, tag="ps", name="st")
                            for qc in range(2):
                                mm(
                                    st[:, qc * 512 : (qc + 1) * 512],
                                    qkT[off : off + D, 3 + g, kb * P : (kb + 1) * P],
                                    qkT[off : off + D, g, hf * HF + qc * 512 : hf * HF + (qc + 1) * 512],
                                    start=True,
                                    stop=True,
                                )
                            if kb * P // HF == hf:
                                dcol = kb * P - hf * HF
                                nc.vector.tensor_mul(
                                    st[:, dcol : dcol + P],
                                    st[:, dcol : dcol + P],
                                    masks[:, h, :],
                                )
                            pt = ptp.tile([P, HF], bf16, tag="pt")
                            nc.scalar.activation(pt[:], st[:], Exp, scale=SCALE)
                            for qc in range(2):
                                mm(
                                    av[:, qc * 512 : (qc + 1) * 512],
                                    vaug[:, kb, h, :],
                                    pt[:, qc * 512 : (qc + 1) * 512],
                                    start=(kb == 0),
                                    stop=(kb == KB - 1),
                                )
                            # PE keep-warm: the attention loop alone is
                            # ACT(exp)-bound at ~80% PE utilization, which
                            # lets the tensor engine drop out of its high
                            # p-state (observed 1.6x slower MMs).  Once the
                            # scheduler's hoistable qkv work is exhausted
                            # (from head 2 on), burn the slack with an
                            # exact no-op: av += 0^T @ pt.
                            if p >= 1 and 1 <= kb <= 14:
                                mm(
                                    av[:, 0:192],
                                    zt[:],
                                    pt[:, 0:192],
                                    start=False,
                                    stop=False,
                                )
                        # normalize: rows 0..63 * recip(row 64).  In-place
                        # single-lane reciprocal + SBUF-source broadcast DMA
                        # keeps the chain short (it gates the proj start via
                        # the last head).
                        un = unp.tile([P, HF], f32, tag="un")
                        nc.vector.tensor_copy(un[0 : D + 1, :], av[:])
                        nc.sync.dma_start(
                            rdram[h, hf * HF : (hf + 1) * HF], un[D : D + 1, :]
                        )
                        rp = rbp.tile([P, P], f32, tag="rp")
                        nc.sync.dma_start(
                            rp[0:8, :],
                            rdram[h, hf * HF : (hf + 1) * HF].rearrange(
                                "(a b) -> a b", a=8
                            ),
                        )
                        nc.vector.reciprocal(rp[0:8, :], rp[0:8, :])
                        nc.sync.dma_start(
                            rdram2[h, hf * HF : (hf + 1) * HF], rp[0:8, :]
                        )
                        rb = rbp.tile([P, HF], f32, tag="rb")
                        nc.sync.dma_start(
                            rb[0:D, :],
                            rdram2[h : h + 1, hf * HF : (hf + 1) * HF]
                            .broadcast_to([D, HF]),
                        )
                        nc.vector.tensor_mul(
                            attnT[off : off + D, g, hf * HF : (hf + 1) * HF],
                            un[0:D, :],
                            rb[0:D, :],
                        )

            # ---- output projection (transposed) ------------------------
            for m in range(CT):
                ot = otp.tile([P, N], bf16, tag="ot")
                for half in range(2):
                    po = pwork.tile([P, HF], f32, tag="ps", name=f"po{m}_{half}")
                    for g3 in range(G3):
                        for qc in range(2):
                            sl = slice(half * HF + qc * 512,
                                       half * HF + (qc + 1) * 512)
                            psl = slice(qc * 512, (qc + 1) * 512)
                            mm(
                                po[:, psl],
                                wp[:, g3, m * P : (m + 1) * P],
                                attnT[:, g3, sl],
                                start=(g3 == 0),
                                stop=(g3 == G3 - 1),
                            )
                    ((nc.scalar.copy if (m + half) % 2 else
                      nc.vector.tensor_copy))(
                        ot[:, half * HF : (half + 1) * HF], po[:]
                    )
                    # stream each output half out as soon as it's copied so
                    # the final DMA tail is ~0.5 MB, not 6 MB.  On the ACT
                    # hwdge queue: the sync queue carries the normalize
                    # broadcasts that gate the proj start.
                    nc.scalar.dma_start(
                        outT[m * P : (m + 1) * P, half * HF : (half + 1) * HF],
                        ot[:, half * HF : (half + 1) * HF],
                    )

    if not nc.is_finalized():
        nc.finalize()
    return nc


def _get_program():
    if "nc" not in _CACHE:
        _CACHE["nc"] = _build_program()
    return _CACHE["nc"]


def _in_maps(x, w_qkv, w_proj, temperature):
    import ml_dtypes

    bf = ml_dtypes.bfloat16
    t = np.asarray(temperature, dtype=np.float32).reshape(H)
    maps = []
    xTs = {}
    for c in range(8):
        b, h0 = c // 2, NH * (c % 2)
        if b not in xTs:
            xTs[b] = np.ascontiguousarray(
                np.asarray(x[b], dtype=np.float32).T
            ).astype(bf)
        cols = slice(D * h0, D * h0 + NH * D)
        wq = np.concatenate(
            [w_qkv[:, cols], w_qkv[:, C:][:, cols], w_qkv[:, 2 * C :][:, cols]],
            axis=1,
        )
        maps.append(
            {
                "xT": xTs[b],
                "wqkv": np.ascontiguousarray(wq, dtype=np.float32).astype(bf),
                "wproj": np.ascontiguousarray(
                    w_proj[D * h0 : D * h0 + NH * D, :], dtype=np.float32
                ).astype(bf),
                "temp": np.ascontiguousarray(
                    np.broadcast_to(t[h0 : h0 + NH].reshape(1, NH), (P, NH))
                ),
            }
        )
    return maps


def _install_profile_hook():
    """The agent image's antenv lacks axon_hooks; synthesize it and register
    the ctypes NTFF hook so run_bass_kernel_spmd(trace=True) can profile."""
    import types, importlib

    if "antenv.axon_hooks" not in sys.modules:
        import antenv

        mod = types.ModuleType("antenv.axon_hooks")
        _state = {"hook": None}
        mod.set_axon_ntff_profile_hook = lambda h: _state.__setitem__("hook", h)
        mod.get_axon_ntff_profile_hook = lambda: _state["hook"]
        sys.modules["antenv.axon_hooks"] = mod
        antenv.axon_hooks = mod
    from antenv.axon_hooks import (
        get_axon_ntff_profile_hook,
        set_axon_ntff_profile_hook,
    )

    if get_axon_ntff_profile_hook() is None:
        tb = importlib.import_module("trn_agent_boot.trn_boot")
        hook = tb._ntff_profile_via_ctypes("/opt/axon/libaxon_pjrt.so")
        set_axon_ntff_profile_hook(hook)


def kernel(x, w_qkv, w_proj, b_proj, temperature, _trace=False):
    from concourse.bass_utils import run_bass_kernel_spmd

    if _trace:
        try:
            _install_profile_hook()
        except Exception as e:  # profiling is best-effort
            print(f"profile hook install failed: {e}")

    nc = _get_program()
    maps = _in_maps(
        np.asarray(x, np.float32),
        np.asarray(w_qkv, np.float32),
        np.asarray(w_proj, np.float32),
        np.asarray(temperature, np.float32),
    )
    res = run_bass_kernel_spmd(nc, maps, list(range(8)), trace=_trace)
    parts = [np.asarray(r["outT"]).astype(np.float32) for r in res.results]
    bp = np.asarray(b_proj, np.float32)
    out = np.stack(
        [(parts[2 * b] + parts[2 * b + 1]).T + bp for b in range(B)]
    ).astype(np.float32)
    if _trace:
        _CACHE["last_result"] = res
    return out


# revision 34
# speedup vs baseline: 1.6225x; 1.3075x over previous
"""LocalitySelfAttention TRN2 kernel.

B=4, N=2048, C=768, H=12, D=64.  8 cores: core c -> batch c//2, heads
6*(c%2) .. 6*(c%2)+6 (6 contiguous heads).  Each core computes its heads'
qkv projection, attention (scores kept transposed: [keys, qrows] so softmax
sums come from a fused ones-column in the AV matmul), and a partial output
projection restricted to its heads' 384 rows of w_proj.  Host sums the two
partials per batch and adds b_proj.

All-transposed dataflow: host passes x[b].T in bf16; q/k are produced
transposed ([64, 2048] per head, stationary = w_qkv columns), v natural
([2048, 64], stationary = xT blocks).  ST block = kT_blk.T @ qT ->
[128 keys, qrows]; exp on ACT with scale=0.125; AV: lhsT = v_aug
[keys, 64+1(ones)], rhs = PT -> outT_aug [65, qrows] accumulated over key
blocks; row 64 = softmax sums.  Diagonal temperature factor: one [128,128]
mask multiply per (head, kblock) on the diagonal sub-block before exp.

Single shared PSUM work pool (3 x 4KB slots) + one AV accumulator slot so
the Tile scheduler can overlap the qkv matmuls of head-pair p+1 and the
output projection with the ACT(exp)-bound attention inner loop.  The exp
on the scalar engine (192 x [128,1024] tiles) is the hard floor; every
other engine's work is arranged to hide under it.
"""

import sys
import numpy as np

if "/opt/trn_rl_repo" not in sys.path:
    sys.path.insert(0, "/opt/trn_rl_repo")

B, N, C, H = 4, 2048, 768, 12
D = C // H          # 64
NH = 6              # heads per core
P = 128
CT = C // P         # 6 contraction tiles
KB = N // P         # 16 key blocks
QC = N // 512       # 4 free-dim chunks of 512
SCALE = float(D) ** -0.5  # 0.125

_CACHE = {}


def _build_program():
    import concourse.bass as bass
    import concourse.mybir as mybir
    import concourse.tile as tile
    from concourse import bacc
    from concourse.masks import make_identity

    f32 = mybir.dt.float32
    bf16 = mybir.dt.bfloat16
    Exp = mybir.ActivationFunctionType.Exp
    mult = mybir.AluOpType.mult
    add = mybir.AluOpType.add

    nc = bacc.Bacc()
    xT = nc.dram_tensor("xT", [C, N], bf16, kind="ExternalInput")
    wqkv = nc.dram_tensor("wqkv", [C, 3 * NH * D], bf16, kind="ExternalInput")
    wproj = nc.dram_tensor("wproj", [NH * D, C], bf16, kind="ExternalInput")
    temp = nc.dram_tensor("temp", [P, NH], f32, kind="ExternalInput")
    outT = nc.dram_tensor("outT", [C, N], bf16, kind="ExternalOutput")
    rdram = nc.dram_tensor("rscratch", [NH, N], f32)  # sum rows bounce
    rdram2 = nc.dram_tensor("rscratch2", [NH, N], f32)  # recip rows bounce

    HF = N // 2  # 1024-column halves
    G3 = NH * D // P  # 3 row-groups of w_proj

    def mm(out, lhsT, rhs, **kw):
        nc.tensor.matmul(out, lhsT, rhs, **kw)

    with tile.TileContext(nc) as tc:
        with (
            tc.tile_pool(name="const", bufs=1) as constp,
            tc.tile_pool(name="persist", bufs=1) as persist,
            tc.tile_pool(name="pwork", bufs=3, space=bass.MemorySpace.PSUM) as pwork,
            tc.tile_pool(name="pav", bufs=1, space=bass.MemorySpace.PSUM) as pav,
            tc.tile_pool(name="pt", bufs=6) as ptp,
            tc.tile_pool(name="rb", bufs=2) as rbp,
            tc.tile_pool(name="un", bufs=2) as unp,
            tc.tile_pool(name="ot", bufs=2) as otp,
        ):
            # ---- inputs ------------------------------------------------
            # inputs: xts on the SP DMA queue, weights on the ACT DMA queue
            # so the first v matmul's operands land in parallel.
            xts, wqs = [], []
            for t in range(CT):
                xti = persist.tile([P, N], bf16, tag=f"xt{t}")
                xts.append(xti)
                wqi = persist.tile([P, 3 * NH * D], bf16, tag=f"wq{t}")
                nc.scalar.dma_start(wqi[:], wqkv[t * P : (t + 1) * P, :])
                wqs.append(wqi)
            # x loads split in column halves, low half first: the first half
            # of the v blocks (and qk half-tiles) can start before the high
            # halves land.
            for half in range(2):
                for t in range(CT):
                    nc.sync.dma_start(
                        xts[t][:, half * HF : (half + 1) * HF],
                        xT[t * P : (t + 1) * P, half * HF : (half + 1) * HF],
                    )
            wp = persist.tile([P, G3, C], bf16, tag="wp")
            for g3 in range(G3):
                nc.scalar.dma_start(wp[:, g3, :], wproj[g3 * P : (g3 + 1) * P, :])

            # ---- setup: temperature diag masks (1 - t_h * I) -----------
            ident = constp.tile([P, P], f32, tag="ident")
            make_identity(nc, ident[:])
            tbc = constp.tile([P, NH], f32, tag="tbc")
            nc.sync.dma_start(tbc[:, :], temp[:, :])
            ntb = constp.tile([P, NH], f32, tag="ntb")
            nc.vector.tensor_scalar_mul(ntb[:, :], tbc[:, :], -1.0)
            masks = constp.tile([P, NH, P], f32, tag="masks")
            for h in range(NH):
                nc.vector.tensor_scalar(
                    masks[:, h, :], ident[:], ntb[:, h : h + 1], 1.0, mult, add
                )

            # persistent: qT/kT (bf16, transposed) and v_aug with ones col
            qkT = persist.tile([P, 2 * NH, N], bf16, tag="qkT")  # 0-2 q, 3-5 k
            vaug = persist.tile([P, KB, NH, D + 1], bf16, tag="vaug")
            onesrc = constp.tile([P, KB * NH], f32, tag="onesrc")
            nc.vector.memset(onesrc[:], 1.0)
            # zero stationary for PE keep-warm dummies (see attention loop)
            zt = constp.tile([P, D + 1], bf16, tag="zt")
            nc.vector.memset(zt[:], 0.0)
            nc.vector.tensor_copy(
                vaug[:, :, :, D : D + 1],
                onesrc[:].rearrange("p (a b c) -> p a b c", a=KB, b=NH),
            )
            attnT = persist.tile([P, G3, N], bf16, tag="attnT")

            # ---- v projection (all heads), then q/k transposed ---------
            # All in the shared ps ring (bufs=3) so MMs pipeline against the
            # PSUM->SBUF copies, which alternate DVE/ACT (ACT is idle until
            # the first exp).
            for rb_i in range(KB):
                psv = pwork.tile([P, NH * D], f32, tag="ps", name=f"psv{rb_i}")
                for t in range(CT):
                    mm(
                        psv[:],
                        xts[t][:, rb_i * P : (rb_i + 1) * P],
                        wqs[t][:, 2 * NH * D : 3 * NH * D],
                        start=(t == 0),
                        stop=(t == CT - 1),
                    )
                (nc.vector.tensor_copy if rb_i % 2 else nc.scalar.copy)(
                    vaug[:, rb_i, :, 0:D],
                    psv[:].rearrange("p (h d) -> p h d", h=NH),
                )
            # head-pair 0's q/k groups LAST-but-first: emit q0/k0 first so
            # the first attention scores' inputs are ready the moment the
            # PE reaches them, keeping the qkv->attention transition
            # seamless in the in-order queue.
            for grp in (0, 3, 1, 4, 2, 5):
                for half in range(2):
                    ps = pwork.tile([P, HF], f32, tag="ps",
                                    name=f"qk{grp}_{half}")
                    for t in range(CT):
                        for qc in range(2):
                            mm(
                                ps[:, qc * 512 : (qc + 1) * 512],
                                wqs[t][:, grp * P : (grp + 1) * P],
                                xts[t][:, half * HF + qc * 512 : half * HF + (qc + 1) * 512],
                                start=(t == 0),
                                stop=(t == CT - 1),
                            )
                    dst = qkT[:, grp, half * HF : (half + 1) * HF]
                    ((nc.scalar.copy if (grp + half) % 2 else
                      nc.vector.tensor_copy))(dst, ps[:])

            # ---- attention, head pair by head pair ---------------------
            for p in range(3):
                for hi in range(2):
                    h = 2 * p + hi
                    g = p
                    off = hi * D
                    for hf in range(2):
                        av = pav.tile([D + 1, HF], f32, tag="av",
                                      name=f"av{h}_{hf}")
                        for kb in range(KB):
                            st = pwork.tile([P, HF], f32, tag="ps", name="st")
                            for qc in range(2):
                                mm(
                                    st[:, qc * 512 : (qc + 1) * 512],
                                    qkT[off : off + D, 3 + g, kb * P : (kb + 1) * P],
                                    qkT[off : off + D, g, hf * HF + qc * 512 : hf * HF + (qc + 1) * 512],
                                    start=True,
                                    stop=True,
                                )
                            if kb * P // HF == hf:
                                dcol = kb * P - hf * HF
                                nc.vector.tensor_mul(
                                    st[:, dcol : dcol + P],
                                    st[:, dcol : dcol + P],
                                    masks[:, h, :],
                                )
                            pt = ptp.tile([P, HF], bf16, tag="pt")
                            nc.scalar.activation(pt[:], st[:], Exp, scale=SCALE)
                            for qc in range(2):
                                mm(
                                    av[:, qc * 512 : (qc + 1) * 512],
                                    vaug[:, kb, h, :],
                                    pt[:, qc * 512 : (qc + 1) * 512],
                                    start=(kb == 0),
                                    stop=(kb == KB - 1),
                                )
                            # PE keep-warm: the attention loop alone is
                            # ACT(exp)-bound at ~80% PE utilization, which
                            # lets the tensor engine drop out of its high
                            # p-state (observed 1.6x slower MMs).  Once the
                            # scheduler's hoistable qkv work is exhausted
                            # (from head 2 on), burn the slack with an
                            # exact no-op: av += 0^T @ pt.
                            if p >= 1 and 1 <= kb <= 14:
                                mm(
                                    av[:, 0:256],
                                    zt[:],
                                    pt[:, 0:256],
                                    start=False,
                                    stop=False,
                                )
                        # normalize: rows 0..63 * recip(row 64).  In-place
                        # single-lane reciprocal + SBUF-source broadcast DMA
                        # keeps the chain short (it gates the proj start via
                        # the last head).
                        un = unp.tile([P, HF], f32, tag="un")
                        nc.vector.tensor_copy(un[0 : D + 1, :], av[:])
                        nc.sync.dma_start(
                            rdram[h, hf * HF : (hf + 1) * HF], un[D : D + 1, :]
                        )
                        rp = rbp.tile([P, P], f32, tag="rp")
                        nc.sync.dma_start(
                            rp[0:8, :],
                            rdram[h, hf * HF : (hf + 1) * HF].rearrange(
                                "(a b) -> a b", a=8
                            ),
                        )
                        nc.vector.reciprocal(rp[0:8, :], rp[0:8, :])
                        nc.sync.dma_start(
                            rdram2[h, hf * HF : (hf + 1) * HF], rp[0:8, :]
                        )
                        rb = rbp.tile([P, HF], f32, tag="rb")
                        nc.sync.dma_start(
                            rb[0:D, :],
                            rdram2[h : h + 1, hf * HF : (hf + 1) * HF]
                            .broadcast_to([D, HF]),
                        )
                        nc.vector.tensor_mul(
                            attnT[off : off + D, g, hf * HF : (hf + 1) * HF],
                            un[0:D, :],
                            rb[0:D, :],
                        )

            # ---- output projection (transposed) ------------------------
            for m in range(CT):
                ot = otp.tile([P, N], bf16, tag="ot")
                for half in range(2):
                    po = pwork.tile([P, HF], f32, tag="ps", name=f"po{m}_{half}")
                    for g3 in range(G3):
                        for qc in range(2):
                            sl = slice(half * HF + qc * 512,
                                       half * HF + (qc + 1) * 512)
                            psl = slice(qc * 512, (qc + 1) * 512)
                            mm(
                                po[:, psl],
                                wp[:, g3, m * P : (m + 1) * P],
                                attnT[:, g3, sl],
                                start=(g3 == 0),
                                stop=(g3 == G3 - 1),
                            )
                    ((nc.scalar.copy if (m + half) % 2 else
                      nc.vector.tensor_copy))(
                        ot[:, half * HF : (half + 1) * HF], po[:]
                    )
                    # stream each output half out as soon as it's copied so
                    # the final DMA tail is ~0.5 MB, not 6 MB.  On the ACT
                    # hwdge queue: the sync queue carries the normalize
                    # broadcasts that gate the proj start.
                    nc.scalar.dma_start(
                        outT[m * P : (m + 1) * P, half * HF : (half + 1) * HF],
                        ot[:, half * HF : (half + 1) * HF],
                    )

    if not nc.is_finalized():
        nc.finalize()
    return nc


def _get_program():
    if "nc" not in _CACHE:
        _CACHE["nc"] = _build_program()
    return _CACHE["nc"]


def _in_maps(x, w_qkv, w_proj, temperature):
    import ml_dtypes

    bf = ml_dtypes.bfloat16
    t = np.asarray(temperature, dtype=np.float32).reshape(H)
    maps = []
    xTs = {}
    for c in range(8):
        b, h0 = c // 2, NH * (c % 2)
        if b not in xTs:
            xTs[b] = np.ascontiguousarray(
                np.asarray(x[b], dtype=np.float32).T
            ).astype(bf)
        cols = slice(D * h0, D * h0 + NH * D)
        wq = np.concatenate(
            [w_qkv[:, cols], w_qkv[:, C:][:, cols], w_qkv[:, 2 * C :][:, cols]],
            axis=1,
        )
        maps.append(
            {
                "xT": xTs[b],
                "wqkv": np.ascontiguousarray(wq, dtype=np.float32).astype(bf),
                "wproj": np.ascontiguousarray(
                    w_proj[D * h0 : D * h0 + NH * D, :], dtype=np.float32
                ).astype(bf),
                "temp": np.ascontiguousarray(
                    np.broadcast_to(t[h0 : h0 + NH].reshape(1, NH), (P, NH))
                ),
            }
        )
    return maps


def _install_profile_hook():
    """The agent image's antenv lacks axon_hooks; synthesize it and register
    the ctypes NTFF hook so run_bass_kernel_spmd(trace=True) can profile."""
    import types, importlib

    if "antenv.axon_hooks" not in sys.modules:
        import antenv

        mod = types.ModuleType("antenv.axon_hooks")
        _state = {"hook": None}
        mod.set_axon_ntff_profile_hook = lambda h: _state.__setitem__("hook", h)
        mod.get_axon_ntff_profile_hook = lambda: _state["hook"]
        sys.modules["antenv.axon_hooks"] = mod
        antenv.axon_hooks = mod
    from antenv.axon_hooks import (
        get_axon_ntff_profile_hook,
        set_axon_ntff_profile_hook,
    )

    if get_axon_ntff_profile_hook() is None:
        tb = importlib.import_module("trn_agent_boot.trn_boot")
        hook = tb._ntff_profile_via_ctypes("/opt/axon/libaxon_pjrt.so")
        set_axon_ntff_profile_hook(hook)


def kernel(x, w_qkv, w_proj, b_proj, temperature, _trace=False):
    from concourse.bass_utils import run_bass_kernel_spmd

    if _trace:
        try:
            _install_profile_hook()
        except Exception as e:  # profiling is best-effort
            print(f"profile hook install failed: {e}")

    nc = _get_program()
    maps = _in_maps(
        np.asarray(x, np.float32),
        np.asarray(w_qkv, np.float32),
        np.asarray(w_proj, np.float32),
        np.asarray(temperature, np.float32),
    )
    res = run_bass_kernel_spmd(nc, maps, list(range(8)), trace=_trace)
    parts = [np.asarray(r["outT"]).astype(np.float32) for r in res.results]
    bp = np.asarray(b_proj, np.float32)
    out = np.stack(
        [(parts[2 * b] + parts[2 * b + 1]).T + bp for b in range(B)]
    ).astype(np.float32)
    if _trace:
        _CACHE["last_result"] = res
    return out
